# revision 30
# baseline (speedup 1.0000x reference)
"""DoRA linear kernel for 8 Trainium2 NeuronCores.

out = (base_output + 2.0 * x @ lora_A^T @ lora_B^T) * magnitude / (||base_weight + 2.0 * lora_B @ lora_A||_row + eps)

Sharding (per the row-parallel hint):
  - tokens (B*S = 8192) data-parallel: 1024 per core (x, base_output, out)
  - base_weight / lora_B / magnitude row-parallel: 512 out_features per core
    (per-row norm fully local; mag_scale allgathered, 16KB collective)
  - lora_A and lora_B replicated for the activation path

Precision: the low-rank delta path (x @ A^T @ B^T and B@A) runs in bf16 on
the PE -- fp32 matmuls are dual-pass (fp32_mode=LOW_HIGH) and 2x slower.
x / base_weight / lora_A / lora_B are pre-cast to bf16 on the host (the
device would round them to bf16 anyway; this halves their HBM traffic).
The base_output add, the norm accumulation (PSUM fp32 + fp32
square-accumulate), and the magnitude rescale stay fp32, so the output
error is dominated by the bf16 rounding of the small LoRA delta only.

Engine / DMA-ring assignment (each engine's instruction stream is FIFO):
  - sync  (SP)  ring: magnitude, W tiles, base halves, output stores
  - scalar(ACT) ring: lora_A/B, x tiles; ACT also copies transpose/xa PSUMs
                      to SBUF and does the norm square-accumulate
  - gpsimd SWDGE:     collective in/out + mag broadcast, 2/8 of the
                      epilogue multiplies
  - vector:           epilogue adds, 6/8 of the multiplies, mag tail math
"""

import sys

sys.path.insert(0, "/opt/trn_rl_repo")

import ml_dtypes
import numpy as np

import concourse.bass as bass  # noqa: F401
import concourse.mybir as mybir
import concourse.tile as tile
from concourse import bacc
from concourse.bass_utils import run_bass_kernel_spmd
from concourse.masks import make_identity

N_CORES = 8
T, D, O, R = 8192, 4096, 4096, 64
T_LOC = T // N_CORES  # 1024 tokens per core
O_SH = O // N_CORES  # 512 weight rows per core
SCALING = 2.0
EPS = 1e-8
F32 = mybir.dt.float32
BF16 = mybir.dt.bfloat16
NP_BF16 = ml_dtypes.bfloat16

N_TB = T_LOC // 128  # 8 token blocks per core
N_OC = O_SH // 128  # 4 o-chunks per core (stage 0)
N_DC512 = D // 512  # 8 d-chunks of 512
N_DC128 = D // 128  # 32 d-chunks of 128

_CACHE: dict = {}


def _emit(nc, tc, aps):
    x_d = aps["x_shard"]
    base_d = aps["base_shard"]
    w_d = aps["w_shard"]
    b_sh_d = aps["b_shard"]
    b_full_d = aps["b_full"]
    a_d = aps["a_full"]
    mag_d = aps["mag_shard"]
    out_d = aps["out_shard"]

    import contextlib

    ctx = contextlib.ExitStack()
    with ctx:
        const = ctx.enter_context(tc.tile_pool(name="const", bufs=1))
        wpool = ctx.enter_context(tc.tile_pool(name="wpool", bufs=2))
        xpool = ctx.enter_context(tc.tile_pool(name="xpool", bufs=3))
        bpool = ctx.enter_context(tc.tile_pool(name="bpool", bufs=2))
        xtpool = ctx.enter_context(tc.tile_pool(name="xtpool", bufs=2))
        xapool = ctx.enter_context(tc.tile_pool(name="xapool", bufs=8))
        opool = ctx.enter_context(tc.tile_pool(name="opool", bufs=5))
        scpool = ctx.enter_context(tc.tile_pool(name="scpool", bufs=2))
        p_u = ctx.enter_context(tc.tile_pool(name="p_u", bufs=2, space="PSUM"))
        p_t = ctx.enter_context(tc.tile_pool(name="p_t", bufs=2, space="PSUM"))
        p_xa = ctx.enter_context(tc.tile_pool(name="p_xa", bufs=1, space="PSUM"))
        p_o = ctx.enter_context(tc.tile_pool(name="p_o", bufs=3, space="PSUM"))
        dram = ctx.enter_context(tc.tile_pool(name="dram", bufs=1, space="DRAM"))

        ident = const.tile([128, 128], BF16)
        make_identity(nc, ident[:])

        x_r = x_d.rearrange("(tb p) d -> tb p d", p=128)
        base_r = base_d.rearrange("(tb p) d -> tb p d", p=128)
        out_r = out_d.rearrange("(tb p) d -> tb p d", p=128)
        w_r = w_d.rearrange("(oc p) d -> oc p d", p=128)

        # ---- phase A: DMA triggers
        # scalar ring: lora tensors (A pre-scaled by 2, B pre-transposed on
        # host -- all contiguous row loads) then x tiles
        a16_sb = const.tile([R, D], BF16)
        nc.scalar.dma_start(a16_sb[:], a_d[:])
        b2ft_sb = const.tile([R, O], BF16)
        nc.scalar.dma_start(b2ft_sb[:], b_full_d[:])
        b2st_sb = const.tile([R, O_SH], BF16)
        nc.scalar.dma_start(b2st_sb[:], b_sh_d[:])

        x_tiles = {}

        def load_x(tb):
            t = xpool.tile([128, D], BF16, tag="x", name=f"x_{tb}")
            nc.scalar.dma_start(t[:], x_r[tb])
            x_tiles[tb] = t

        load_x(0)
        load_x(1)
        load_x(2)

        # sync ring: magnitude, W, base halves (stores appended per-tb later)
        magsh_sb = const.tile([128, 4], F32)
        nc.sync.dma_start(magsh_sb[:], mag_d.rearrange("(oc p) -> p oc", p=128))
        w_tiles = []
        for oc in range(N_OC):
            wt = wpool.tile([128, D], BF16, tag="w", name=f"w_{oc}")
            nc.sync.dma_start(wt[:], w_r[oc])
            w_tiles.append(wt)
        base_tiles = {}
        for tb in range(N_TB):
            bt = bpool.tile([128, D], F32, tag="base", name=f"base_{tb}")
            nc.sync.dma_start(bt[:], base_r[tb])
            base_tiles[tb] = bt

        # ---- phase B: preprocessing transposes (bf16)
        at_sb = const.tile([128, 64 * N_DC128], BF16)
        for g in range(2):
            pt = p_t.tile([128, 1024], BF16, tag="pt", name=f"pta_{g}")
            for j in range(16):
                dc = 16 * g + j
                nc.tensor.transpose(
                    pt[:, 64 * j : 64 * (j + 1)],
                    a16_sb[:, 128 * dc : 128 * (dc + 1)],
                    ident[0:R, 0:R],
                )
            nc.scalar.copy(at_sb[:, 1024 * g : 1024 * (g + 1)], pt[:])


        # ---- stage 0: ||W + 2 B A||^2 rows, then mag_scale + allgather
        ss_sb = const.tile([128, N_OC, N_DC512], F32)
        magsc_sb = const.tile([128, 4], F32)
        magb_sb = const.tile([128, O], F32)

        for oc in range(N_OC):
            for dc in range(N_DC512):
                pu = p_u.tile([128, 512], F32, tag="pu", name=f"pu_{oc}_{dc}")
                nc.tensor.matmul(
                    pu[:],
                    b2st_sb[:, 128 * oc : 128 * (oc + 1)],
                    a16_sb[:, 512 * dc : 512 * (dc + 1)],
                    start=True,
                    stop=False,
                )
                nc.tensor.matmul(
                    pu[:],
                    ident[:],
                    w_tiles[oc][:, 512 * dc : 512 * (dc + 1)],
                    start=False,
                    stop=True,
                )
                sq = scpool.tile([128, 512], BF16, tag="sq", name=f"sq_{oc}_{dc}")
                nc.scalar.activation(
                    sq[:],
                    pu[:],
                    mybir.ActivationFunctionType.Square,
                    accum_out=ss_sb[:, oc, dc : dc + 1],
                )
        def emit_mag_tail_and_collective():
            for oc in range(N_OC):
                ssum = scpool.tile([128, 1], F32, tag="ssum", name=f"ssum_{oc}")
                nc.vector.tensor_reduce(
                    ssum[:],
                    ss_sb[:, oc, :],
                    axis=mybir.AxisListType.X,
                    op=mybir.AluOpType.add,
                )
                nrm = scpool.tile([128, 1], F32, tag="nrm", name=f"nrm_{oc}")
                nc.scalar.sqrt(nrm[:], ssum[:])
                nc.vector.tensor_scalar_add(nrm[:], nrm[:], EPS)
                rinv = scpool.tile([128, 1], F32, tag="rinv", name=f"rinv_{oc}")
                nc.vector.reciprocal(rinv[:], nrm[:])
                nc.vector.tensor_tensor(
                    out=magsc_sb[:, oc : oc + 1],
                    in0=rinv[:],
                    in1=magsh_sb[:, oc : oc + 1],
                    op=mybir.AluOpType.mult,
                )
            cc_in = dram.tile([O_SH], F32)
            cc_out = dram.tile([O], F32, addr_space="Shared")
            nc.gpsimd.dma_start(cc_in.rearrange("(oc p) -> p oc", p=128), magsc_sb[:])
            nc.gpsimd.collective_compute(
                "AllGather",
                mybir.AluOpType.bypass,
                replica_groups=[list(range(N_CORES))],
                ins=[cc_in[:]],
                outs=[cc_out[:]],
            )
            nc.sync.dma_start(magb_sb[:], cc_out[None, :].partition_broadcast(128))

        # ---- main-loop helpers
        def emit_stage1(tb):
            """xa^T[64, 128] = A @ x_tb^T via PE-transposed bf16 x chunks."""
            pxa = p_xa.tile([R, 128], F32, tag="pxa", name=f"pxa_{tb}")
            xh = x_tiles.pop(tb)
            for g in range(4):
                pt = p_t.tile([128, 1024], BF16, tag="pt", name=f"ptx_{tb}_{g}")
                for j in range(8):
                    nc.tensor.transpose(
                        pt[:, 128 * j : 128 * (j + 1)],
                        xh[:, 128 * (8 * g + j) : 128 * (8 * g + j + 1)],
                        ident[:],
                    )
                xt = xtpool.tile([128, 1024], BF16, tag="xt", name=f"xt_{tb}_{g}")
                nc.scalar.copy(xt[:], pt[:])
                for j in range(8):
                    dc = 8 * g + j
                    nc.tensor.matmul(
                        pxa[:],
                        at_sb[:, 64 * dc : 64 * (dc + 1)],
                        xt[:, 128 * j : 128 * (j + 1)],
                        start=(dc == 0),
                        stop=(dc == N_DC128 - 1),
                    )
            xa_sb = xapool.tile([R, 128], BF16, tag="xa", name=f"xa_{tb}")
            nc.scalar.copy(xa_sb[:], pxa[:])
            return xa_sb

        osb_tiles = {}

        def emit_stage2_adds(tb, xa_sb):
            """delta matmuls + base add into the output tile (no mag yet)."""
            osb = opool.tile([128, D], F32, tag="o", name=f"osb_{tb}")
            osb_tiles[tb] = osb
            for h in range(2):
                pos = [
                    p_o.tile([128, 512], F32, tag="po", name=f"po_{tb}_{h}_{j}")
                    for j in range(4)
                ]
                for j in range(4):
                    och = 4 * h + j
                    nc.tensor.matmul(
                        pos[j][:],
                        xa_sb[:],
                        b2ft_sb[:, 512 * och : 512 * (och + 1)],
                        start=True,
                        stop=True,
                    )
                bh = base_tiles[tb]
                for j in range(4):
                    och = 4 * h + j
                    nc.vector.tensor_tensor(
                        out=osb[:, 512 * och : 512 * (och + 1)],
                        in0=pos[j][:],
                        in1=bh[:, 512 * och : 512 * (och + 1)],
                        op=mybir.AluOpType.add,
                    )
                if h == 1:
                    del base_tiles[tb]

        def emit_mults_and_store(tb):
            """magnitude rescale in-place (wide tiles, DVE + GpSimd) + store."""
            osb = osb_tiles[tb]
            for h in range(2):
                eng = nc.gpsimd if (h == 1 and tb % 2 == 0) else nc.vector
                eng.tensor_tensor(
                    out=osb[:, 2048 * h : 2048 * (h + 1)],
                    in0=osb[:, 2048 * h : 2048 * (h + 1)],
                    in1=magb_sb[:, 2048 * h : 2048 * (h + 1)],
                    op=mybir.AluOpType.mult,
                )
            nc.sync.dma_start(out_r[tb], osb[:])

        # ---- phase C: main loop; mag tail after tb2, mults deferred by 4
        for tb in range(N_TB):
            if tb + 3 < N_TB:
                load_x(tb + 3)
            xa_sb = emit_stage1(tb)
            emit_stage2_adds(tb, xa_sb)
            if tb == 2:
                emit_mag_tail_and_collective()
            if tb >= 4:
                emit_mults_and_store(tb - 4)
        for tb in range(N_TB - 4, N_TB):
            emit_mults_and_store(tb)


def _build():
    nc = bacc.Bacc(
        "TRN2", target_bir_lowering=False, debug=False, num_devices=N_CORES
    )
    aps = {
        "x_shard": nc.dram_tensor("x_shard", [T_LOC, D], BF16, kind="ExternalInput").ap(),
        "base_shard": nc.dram_tensor(
            "base_shard", [T_LOC, O], F32, kind="ExternalInput"
        ).ap(),
        "w_shard": nc.dram_tensor("w_shard", [O_SH, D], BF16, kind="ExternalInput").ap(),
        "b_shard": nc.dram_tensor("b_shard", [R, O_SH], BF16, kind="ExternalInput").ap(),
        "b_full": nc.dram_tensor("b_full", [R, O], BF16, kind="ExternalInput").ap(),
        "a_full": nc.dram_tensor("a_full", [R, D], BF16, kind="ExternalInput").ap(),
        "mag_shard": nc.dram_tensor(
            "mag_shard", [O_SH], F32, kind="ExternalInput"
        ).ap(),
        "out_shard": nc.dram_tensor(
            "out_shard", [T_LOC, O], F32, kind="ExternalOutput"
        ).ap(),
    }
    with tile.TileContext(nc) as tc:
        _emit(nc, tc, aps)
    nc.compile()
    return nc


def run(inputs: dict, trace: bool = False):
    """Run the SPMD kernel on full inputs; returns (full_output, BassKernelResults)."""
    if "nc" not in _CACHE:
        _CACHE["nc"] = _build()
    nc = _CACHE["nc"]

    x = np.asarray(inputs["x"], dtype=np.float32).reshape(T, D).astype(NP_BF16)
    base = np.asarray(inputs["base_output"], dtype=np.float32).reshape(T, O)
    w = np.asarray(inputs["base_weight"], dtype=np.float32).astype(NP_BF16)
    a = np.ascontiguousarray(
        (np.asarray(inputs["lora_A"], dtype=np.float32) * SCALING).astype(NP_BF16)
    )
    bt = np.asarray(inputs["lora_B"], dtype=np.float32).astype(NP_BF16).T
    mag = np.asarray(inputs["magnitude"], dtype=np.float32)

    in_maps = []
    for c in range(N_CORES):
        in_maps.append(
            {
                "x_shard": np.ascontiguousarray(x[c * T_LOC : (c + 1) * T_LOC]),
                "base_shard": np.ascontiguousarray(base[c * T_LOC : (c + 1) * T_LOC]),
                "w_shard": np.ascontiguousarray(w[c * O_SH : (c + 1) * O_SH]),
                "b_shard": np.ascontiguousarray(bt[:, c * O_SH : (c + 1) * O_SH]),
                "b_full": np.ascontiguousarray(bt),
                "a_full": a,
                "mag_shard": np.ascontiguousarray(mag[c * O_SH : (c + 1) * O_SH]),
            }
        )

    res = run_bass_kernel_spmd(
        nc, in_maps, core_ids=list(range(N_CORES)), trace=trace
    )
    out = np.concatenate(
        [res.results[c]["out_shard"] for c in range(N_CORES)], axis=0
    )
    return out, res


def kernel(**inputs) -> np.ndarray:
    x = inputs["x"]
    out, _ = run(inputs)
    return out.reshape(x.shape[0], x.shape[1], O).astype(np.float32)
